# revision 24
# baseline (speedup 1.0000x reference)
"""DenseCapsule (dynamic routing) kernel for 8x Trainium2 NeuronCores.

Sharding: in_num_caps I=4608 split across 8 cores (576 each, zero-padded to
640 = 5 partition tiles); every core keeps f16 copies of its x/w shard in
SBUF and recomputes x_hat contributions on the PE. Routing iterations:

  iter0   S0 = 0.1 * sum_i x_hat   (softmax of zero logits is uniform) via
          40 accumulating PE matmuls; 80KB AllReduce; squash on-chip.
  iter1/2 logit pass: Wu[i',k,b] = sum_d w*u via the block-diagonal uZ
          trick (contraction over (d,k)=128 partitions, 2 N=512 matmuls
          per (o,t)); PSUM drained by Scalar to f16; DVE multiplies by x
          and k-reduces via halving adds (GpSimd takes 1 in 5 tiles);
          softmax folds 1/Z into Rx = x*R so c is never materialized
          (cx = E_o * Rx); weighted pass: per (o,t,k) accumulating
          [128,16]x[128,128] matmuls into per-o PSUM.

Optimizations vs the naive pipeline:
  - iteration-2 logits use exp(b1+delta) = exp(b1)*exp(delta), so the
    running-logit read-modify-write chain disappears (E *= exp(delta)).
  - dep-free "warm" matmuls keep the PE HAM clock un-throttled across the
    AllReduce waits.
  - t-outer logit loop hides softmax(t) under logit(t+1); per-t input DMA
    slicing lets iter0 start before the full 6.5MB load completes.
  - squash uses sig = n/(1+n^2) (the 1e-8 eps is negligible) with a
    single Scalar sqrt.

kernel(**inputs) takes FULL x/weight, shards on host, runs the SPMD
program on cores 0-7 (two 80KB AllReduces on-device; the final per-core
S2 partials are summed + squashed on host), and returns [128,10,4,4].
A retry guard re-runs the program if the output is non-finite (rare
transient transfer corruption was observed once).
"""

import sys, os
if '/opt/trn_rl_repo' not in sys.path:
    sys.path.insert(0, '/opt/trn_rl_repo')
import numpy as np

import concourse.bass as bass
import concourse.bacc as bacc
import concourse.tile as tile
import concourse.mybir as mybir
from concourse import bass_utils

F32 = mybir.dt.float32
F16 = mybir.dt.float16
AF = mybir.ActivationFunctionType
ALU = mybir.AluOpType
AX = mybir.AxisListType

B, I, K, O, D = 128, 4608, 8, 10, 16
NCORES = 8
ISH = I // NCORES
NT = 5
IPAD = NT * 128
OD = O * D
ODB = O * B


def build_program(stage=4):
    nc = bacc.Bacc("TRN2", target_bir_lowering=False, debug=False,
                   num_devices=NCORES)

    xkb_d = nc.dram_tensor("xkb", [128, NT, K, B], F16, kind="ExternalInput").ap()
    wkodb_d = nc.dram_tensor("wkodb", [128, NT, K, O, D], F16, kind="ExternalInput").ap()
    wdT_d = nc.dram_tensor("wdT", [128, O, NT, 128], F16, kind="ExternalInput").ap()
    id_d = nc.dram_tensor("ident", [128, 128], F32, kind="ExternalInput").ap()

    f01_d = nc.dram_tensor("f01", [128, OD], F32, kind="ExternalOutput").ap()
    s2pT_d = nc.dram_tensor("s2pT", [16, ODB], F32, kind="ExternalOutput").ap()

    with tile.TileContext(nc) as tc:
        with (
            tc.tile_pool(name="big", bufs=1) as big,
            tc.tile_pool(name="work", bufs=10) as work,
            tc.tile_pool(name="wk1", bufs=3) as wk1,
            tc.tile_pool(name="ypool", bufs=3) as ypool,
            tc.tile_pool(name="small", bufs=1) as small,
            tc.tile_pool(name="psA", bufs=2, space="PSUM") as psA,
            tc.tile_pool(name="psB", bufs=2, space="PSUM") as psB,
            tc.tile_pool(name="psC", bufs=1, space="PSUM") as psC,
            tc.tile_pool(name="psD", bufs=1, space="PSUM") as psD,
            tc.tile_pool(name="dram", bufs=2, space="DRAM") as dram,
        ):
            # ---- resident ----
            xkb = big.tile([128, NT, K, B], F16, tag="xkb")
            wkodb = big.tile([128, NT, K, O, D], F16, tag="wkodb")
            wdT = big.tile([128, O, NT, 128], F16, tag="wdT")
            ident = big.tile([128, 128], F32, tag="ident")
            L = big.tile([128, NT, O, B], F16, tag="L")
            E = big.tile([128, NT, O, B], F16, tag="E")
            Rx = big.tile([128, NT, K, B], F16, tag="Rx")
            uZ = big.tile([128, O, K, B], F16, tag="uZ")
            Sfull = big.tile([128, O, D], F32, tag="Sfull")
            u_t = big.tile([128, O, D], F32, tag="u")
            f01 = big.tile([128, O, D], F32, tag="f01")
            spT = big.tile([16, O, B], F32, tag="spT")

            for t in range(NT):
                nc.sync.dma_start(xkb[:, t], xkb_d[:, t])
                nc.sync.dma_start(wkodb[:, t], wkodb_d[:, t])
            nc.sync.dma_start(ident[:], id_d[:])
            nc.scalar.dma_start(wdT[:], wdT_d[:])
            nc.vector.memset(uZ[:], 0.0)

            def all_reduce(src_ap, shape):
                bin_ = dram.tile(shape, F32, tag="arin")
                bout = dram.tile(shape, F32, tag="arout")
                nc.sync.dma_start(bin_[:], src_ap)
                nc.gpsimd.collective_compute(
                    "AllReduce", ALU.add,
                    replica_groups=[list(range(NCORES))],
                    ins=[bin_.opt()], outs=[bout.opt()],
                )
                return bout

            def squash_into_u(S_ap, pre_scale):
                s_sc = small.tile([128, O, D], F32, tag="s_sc")
                nc.vector.tensor_scalar_mul(s_sc[:], S_ap, float(pre_scale))
                sq = small.tile([128, O, D], F32, tag="sq")
                nc.vector.tensor_mul(sq[:], s_sc[:], s_sc[:])
                n2 = small.tile([128, O], F32, tag="n2")
                nc.vector.reduce_sum(n2[:], sq[:], axis=AX.X)
                n1 = small.tile([128, O], F32, tag="n1")
                nc.scalar.sqrt(n1[:], n2[:])
                den = small.tile([128, O], F32, tag="den")
                nc.scalar.add(den[:], n2[:], 1.0)
                rden = small.tile([128, O], F32, tag="rden")
                nc.vector.reciprocal(rden[:], den[:])
                sig = small.tile([128, O], F32, tag="sig")
                nc.vector.tensor_mul(sig[:], n1[:], rden[:])
                sig_b = sig[:].unsqueeze(2).broadcast_to([128, O, D])
                nc.vector.tensor_mul(u_t[:], s_sc[:], sig_b)

            def build_uZ():
                """uZ[16k+d, o, k, b] = u_t[b, o, d]; other rows stay 0."""
                for o in range(O):
                    pt = psC.tile([16, 128], F32, tag="psc")
                    nc.tensor.matmul(pt[:], u_t[:, o, :], ident[:], is_transpose=True)
                    nc.scalar.copy(uZ[0:16, o, 0, :], pt[:])
                for k in range(1, K):
                    eng = nc.scalar if k % 2 == 0 else nc.sync
                    eng.dma_start(uZ[16*k:16*k+16, :, k, :],
                                  uZ[0:16, :, 0, :])

            def logit_tile(o, t, first, variant):
                wps = psA.tile([128, K, B], F32, tag="What")
                nc.tensor.matmul(
                    wps[:].rearrange("p k b -> p (k b)")[:, 0:4*B],
                    wdT[:, o, t, :], uZ[:, o, 0:4, :])
                nc.tensor.matmul(
                    wps[:].rearrange("p k b -> p (k b)")[:, 4*B:8*B],
                    wdT[:, o, t, :], uZ[:, o, 4:8, :])
                # Variant A: DVE drain+mul fused from PSUM, DVE reduce.
                # Variant B: Scalar drain, DVE mul+reduce.
                # Variant C: Scalar drain, DVE mul, GpSimd reduce.
                P = work.tile([128, K, B], F16, tag="P")
                if variant == 0:
                    nc.vector.tensor_mul(P[:], wps[:], xkb[:, t])
                else:
                    Pc = work.tile([128, K, B], F16, tag="Pc")
                    nc.scalar.copy(Pc[:], wps[:])
                    nc.vector.tensor_mul(P[:], Pc[:], xkb[:, t])
                eng = nc.gpsimd if variant == 2 else nc.vector
                eng.tensor_add(P[:, 0:4], P[:, 0:4], P[:, 4:8])
                eng.tensor_add(P[:, 0:2], P[:, 0:2], P[:, 2:4])
                eng.tensor_add(L[:, t, o, :], P[:, 0], P[:, 1])

            def softmax_t(t, first=True):
                # E = exp(L) (pass 1) or E *= exp(L_delta) (pass 2, since
                # exp(b1 + delta) = exp(b1) * exp(delta)); then Z = sum_o E,
                # R = 1/Z via Ln+Exp, Rx = x*R.
                if first:
                    nc.scalar.activation(E[:, t], L[:, t], AF.Exp)
                else:
                    Ex = wk1.tile([128, O, B], F16, tag="Ex")
                    nc.scalar.activation(Ex[:], L[:, t], AF.Exp)
                    nc.gpsimd.tensor_mul(E[:, t], E[:, t], Ex[:])
                Z = wk1.tile([128, B], F32, tag="Z")
                nc.vector.reduce_sum(Z[:], E[:, t].transpose([0, 2, 1]),
                                     axis=AX.X)
                lnz = wk1.tile([128, B], F16, tag="lnz")
                nc.scalar.activation(lnz[:], Z[:], AF.Ln)
                R = wk1.tile([128, B], F16, tag="R")
                nc.scalar.activation(R[:], lnz[:], AF.Exp, scale=-1.0)
                R_b = R[:].unsqueeze(1).broadcast_to([128, K, B])
                nc.vector.tensor_mul(Rx[:, t], xkb[:, t], R_b)

            def pe_warm(n):
                # Dep-free matmuls that keep the PE HAM un-throttled while
                # the AllReduce + squash glue runs.  They sit on the tensor
                # queue before the next pass's matmuls and execute during
                # the collective wait.
                for _ in range(n):
                    dps = psD.tile([128, 128], F32, tag="warmmm")
                    nc.tensor.matmul(dps[:], wdT[:, 0, 0, :], xkb[:, 0, 0, :])

            def logit_softmax_pass(first):
                # B-tiles (Scalar drain + DVE mul/reduce) with isolated
                # C-tiles (GpSimd reduce) to avoid GpSimd clustering.
                for t in range(NT):
                    for o in range(O):
                        variant = 2 if (o + t) % 5 == 3 else 1
                        logit_tile(o, t, first, variant)
                    softmax_t(t, first)

            def weighted_pass():
                for o in range(O):
                    ps = psB.tile([16, 128], F32, tag="s_acc")
                    y = ypool.tile([128, NT, K, B], F16, tag="y")
                    e_b = E[:, :, o, :].unsqueeze(2).broadcast_to(
                        [128, NT, K, B])
                    nc.vector.tensor_mul(y[:], Rx[:], e_b)
                    for t in range(NT):
                        for k in range(K):
                            nc.tensor.matmul(
                                ps[:], wkodb[:, t, k, o, :], y[:, t, k, :],
                                start=(t == 0 and k == 0),
                                stop=(t == NT - 1 and k == K - 1))
                    nc.scalar.copy(spT[:, o, :], ps[:])

            # ================= iteration 0 =================
            ps0t = psA.tile([128, K, B], F32, tag="What")
            ps0 = ps0t[:].rearrange("p k b -> p (k b)")[:, 0:OD]
            for t in range(NT):
                for k in range(K):
                    nc.tensor.matmul(
                        ps0, xkb[:, t, k, :],
                        wkodb[:, t, k].rearrange("p o d -> p (o d)"),
                        start=(t == 0 and k == 0),
                        stop=(t == NT - 1 and k == K - 1))
            sp0 = small.tile([128, OD], F32, tag="sp0")
            nc.scalar.copy(sp0[:], ps0)
            bout0 = all_reduce(sp0[:], [128, OD])
            pe_warm(60)
            nc.sync.dma_start(Sfull[:].rearrange("p o d -> p (o d)"), bout0[:])
            squash_into_u(Sfull[:], 0.1)
            nc.vector.tensor_scalar_mul(f01[:], u_t[:], 0.3)
            build_uZ()

            if stage < 4:
                nc.vector.memset(spT[:], 0.0)

            # ================= iteration 1 =================
            if stage >= 2:
                logit_softmax_pass(first=True)
            if stage >= 3:
                weighted_pass()
                bout1 = all_reduce(spT[:].rearrange("p o b -> p (o b)"), [16, ODB])
                pe_warm(40)
                nc.sync.dma_start(spT[:].rearrange("p o b -> p (o b)"), bout1[:])
                for o in range(O):
                    pt2 = psC.tile([128, 16], F32, tag="psc")
                    nc.tensor.matmul(pt2[:], spT[:, o, :], ident[0:16, 0:16],
                                     is_transpose=True)
                    nc.scalar.copy(Sfull[:, o, :], pt2[:])
                squash_into_u(Sfull[:], 1.0)
                nc.vector.scalar_tensor_tensor(
                    f01[:], u_t[:], 0.3, f01[:], op0=ALU.mult, op1=ALU.add)
                build_uZ()

            # ================= iteration 2 =================
            if stage >= 4:
                logit_softmax_pass(first=False)
                weighted_pass()

            nc.sync.dma_start(f01_d[:], f01[:].rearrange("p o d -> p (o d)"))
            nc.sync.dma_start(s2pT_d[:], spT[:].rearrange("p o b -> p (o b)"))

    nc.compile()
    return nc


def prep_core_inputs(x, w, core):
    xs = x[:, core * ISH:(core + 1) * ISH, :].astype(np.float32)
    ws = w[:, core * ISH:(core + 1) * ISH].astype(np.float32)
    xsp = np.zeros((B, IPAD, K), np.float32); xsp[:, :ISH] = xs
    wsp = np.zeros((O, IPAD, D, K), np.float32); wsp[:, :ISH] = ws

    xT = xsp.reshape(B, NT, 128, K).transpose(2, 1, 3, 0).copy()        # [p,t,k,b]
    wk = wsp.reshape(O, NT, 128, D, K).transpose(2, 1, 4, 0, 3).copy()  # [p,t,k,o,d]
    wdT = np.zeros((128, O, NT, 128), np.float32)
    wtmp = wsp.reshape(O, NT, 128, D, K)
    for k in range(K):
        wdT[16 * k:16 * k + 16] = wtmp[:, :, :, :, k].transpose(3, 0, 1, 2)
    return {
        "xkb": xT.astype(np.float16),
        "wkodb": wk.astype(np.float16),
        "wdT": wdT.astype(np.float16),
        "ident": np.eye(128, dtype=np.float32),
    }


def host_epilogue(f01_core0, s2pT_list):
    s2T = np.sum(np.stack(s2pT_list), axis=0, dtype=np.float32)
    s2 = s2T.reshape(D, O, B).transpose(2, 1, 0).astype(np.float32)
    n = np.linalg.norm(s2, axis=-1, keepdims=True).astype(np.float32)
    scale = (n ** 2 / (1.0 + n ** 2) / (n + 1e-8)).astype(np.float32)
    u2 = (scale * s2).astype(np.float32)
    out = f01_core0.reshape(B, O, D).astype(np.float32) + np.float32(0.4) * u2
    return out.reshape(B, O, 4, 4).astype(np.float32)


def run(x, w, nc=None, trace=False, tmpdir=None):
    if nc is None:
        nc = build_program()
    in_maps = [prep_core_inputs(x, w, c) for c in range(NCORES)]
    res = bass_utils.run_bass_kernel_spmd(
        nc, in_maps, core_ids=list(range(NCORES)), trace=trace, tmpdir=tmpdir)
    out = host_epilogue(res.results[0]["f01"],
                        [res.results[c]["s2pT"] for c in range(NCORES)])
    return out, res


_NC_CACHE = {}

def _get_program():
    if "nc" not in _NC_CACHE:
        _NC_CACHE["nc"] = build_program()
    return _NC_CACHE["nc"]


def kernel(x, weight):
    x = np.asarray(x, dtype=np.float32)
    w = np.asarray(weight, dtype=np.float32)
    out, _ = run(x, w, nc=_get_program())
    for _retry in range(2):
        if np.isfinite(out).all() and np.abs(out).max() < 1e3:
            break
        out, _ = run(x, w, nc=_get_program())
    return out
